# revision 1
# baseline (speedup 1.0000x reference)
"""Trainium2 Bass kernel for nn_MockAttentionHead.

Math note: the reference's final steps are
    scores = softmax(sims*temp); scores *= scale; scores /= (rowsum(scores)+eps)
Since softmax rows sum to 1, the scale multiplication cancels in the final
renormalization up to ~eps/scale ~ 1e-10 relative, so the output equals
exp(temp*sims) row-normalized.  The entire score_dists / input_dists / scale
computation has no effect on the output beyond 1e-7 (verified numerically vs
the jax reference: max rel err 1.4e-6, fp32 noise level).

The [B,D,D] metric tensors also reduce analytically: for m = qq^T/D + I,
  fro = sqrt((s/D+1)^2 + D-1),  q^T m q = s*t  (t = s/D+1, s = ||q||^2),
so norm = sqrt(s*t/fro), and ||xn||^2 = s/norm^2 = fro/t.

Sharding: data-parallel over query rows; 512 rows per core, key side
replicated.  No collectives.
"""

import sys
import numpy as np

sys.path.insert(0, "/opt/trn_rl_repo")

import concourse.bass as bass
import concourse.mybir as mybir
import concourse.tile as tile
from concourse.masks import make_identity

B = 4096
D = 128
NCORES = 8
R = B // NCORES          # 512 query rows per core
IT = R // 128            # 4 i-tiles per core
JTS = B // 128           # 32 j-tiles (128 wide)
KG = 8                   # k-groups of 4 j-tiles (512 wide)
CHUNKS = [(0, 1536), (1536, 1536), (3072, 1024)]   # ragged psum chunks
TEMP = float(np.sqrt(float(D)))
USE_POW = False          # pow/divide are not in the DVE ISA (HW-verified)
OUT_DMA_SPLIT = 4        # output DMAs per i-tile (queue striping)

F32 = mybir.dt.float32
BF16 = mybir.dt.bfloat16
MUL = mybir.AluOpType.mult
ADD = mybir.AluOpType.add
POW = mybir.AluOpType.pow
AX_X = mybir.AxisListType.X
SQRT = mybir.ActivationFunctionType.Sqrt
EXPF = mybir.ActivationFunctionType.Exp
COPYF = mybir.ActivationFunctionType.Copy


def _bcast4(src, col0):
    """[128,4,128] read AP over src[:, col0:col0+4] with the last dim
    broadcast (step 0): value j repeated 128x along free."""
    pstep, pcount = src.ap[0]
    return bass.AP(tensor=src.tensor, offset=src.offset + col0,
                   ap=[[pstep, pcount], [1, 4], [0, 128]])


def _norm_chain(nc, pool, s, n, cD1, label):
    """Metric-norm chain on packed [128, n] row-norm tile `s`.
    Returns (u = 1/norm, a = ||xn||^2 = fro/t).  The reference's +eps
    terms are dropped: they perturb results at the 1e-9 level."""
    t = pool.tile([128, n], F32, name=f"t_{label}", tag=f"t_{label}")
    nc.vector.tensor_scalar(t, s, 1.0 / D, 1.0, MUL, ADD)          # t = s/D+1
    t2 = pool.tile([128, n], F32, name=f"t2_{label}", tag=f"t2_{label}")
    nc.vector.tensor_mul(t2, t, t)
    fro = pool.tile([128, n], F32, name=f"fro_{label}", tag=f"fro_{label}")
    nc.scalar.activation(fro, t2, SQRT, bias=cD1[:, 0:1])          # sqrt(t^2+D-1)
    rec = pool.tile([128, n], F32, name=f"rec_{label}", tag=f"rec_{label}")
    nc.vector.reciprocal(rec, fro)
    rt_ = pool.tile([128, n], F32, name=f"rt_{label}", tag=f"rt_{label}")
    nc.vector.reciprocal(rt_, t)
    a = pool.tile([128, n], F32, name=f"a_{label}", tag=f"a_{label}")
    nc.vector.tensor_mul(a, fro, rt_)                              # fro/t
    num = pool.tile([128, n], F32, name=f"num_{label}", tag=f"num_{label}")
    nc.vector.tensor_mul(num, s, t)                                # s*t
    nc.vector.tensor_mul(num, num, rec)                            # s*t/fro
    qn = pool.tile([128, n], F32, name=f"qn_{label}", tag=f"qn_{label}")
    nc.scalar.activation(qn, num, SQRT)                            # metric norm
    u = pool.tile([128, n], F32, name=f"u_{label}", tag=f"u_{label}")
    nc.vector.reciprocal(u, qn)                                    # 1/norm
    return u, a


def _trace(nc, with_bias, reps=1):
    from contextlib import ExitStack

    qT = nc.dram_tensor("qT", [D, R], F32, kind="ExternalInput").ap()
    kT = nc.dram_tensor("kT", [D, B], F32, kind="ExternalInput").ap()
    wqT = nc.dram_tensor("wqT", [D, D], F32, kind="ExternalInput").ap()
    wkT = nc.dram_tensor("wkT", [D, D], F32, kind="ExternalInput").ap()
    if with_bias:
        bq_row = nc.dram_tensor("bq_row", [1, D], F32, kind="ExternalInput").ap()
        bk_row = nc.dram_tensor("bk_row", [1, D], F32, kind="ExternalInput").ap()
    out = nc.dram_tensor("out", [R, B], F32, kind="ExternalOutput").ap()

    with tile.TileContext(nc) as tc, ExitStack() as ctx:
        consts = ctx.enter_context(tc.tile_pool(name="consts", bufs=1))
        work = ctx.enter_context(tc.tile_pool(name="work", bufs=1))
        scratch = ctx.enter_context(tc.tile_pool(name="scratch", bufs=3))
        ps_small = ctx.enter_context(
            tc.tile_pool(name="ps_small", bufs=2, space="PSUM"))
        ps_main = ctx.enter_context(
            tc.tile_pool(name="ps_main", bufs=2, space="PSUM"))

        ident = consts.tile([128, 128], F32, name="ident")
        make_identity(nc, ident)
        ones2 = consts.tile([2, 128], BF16, name="ones2")
        nc.vector.memset(ones2, 1.0)
        cD1 = consts.tile([128, 1], F32, name="cD1")
        nc.vector.memset(cD1, float(D - 1))

        # q-side inputs first so q projections start immediately
        qT_s = consts.tile([D, R], F32, name="qT_s")
        nc.sync.dma_start(out=qT_s, in_=qT)
        wq_s = consts.tile([D, D], F32, name="wq_s")
        nc.sync.dma_start(out=wq_s, in_=wqT)
        wk_s = consts.tile([D, D], F32, name="wk_s")
        nc.sync.dma_start(out=wk_s, in_=wkT)
        kT_s = consts.tile([D, B], F32, name="kT_s")
        for h in range(4):
            nc.sync.dma_start(out=kT_s[:, h * 1024:(h + 1) * 1024],
                              in_=kT[:, h * 1024:(h + 1) * 1024])
        if with_bias:
            ones1 = consts.tile([1, 128], F32, name="ones1")
            nc.vector.memset(ones1, 1.0)
            bq_s = consts.tile([1, D], F32, name="bq_s")
            nc.sync.dma_start(out=bq_s, in_=bq_row)
            bk_s = consts.tile([1, D], F32, name="bk_s")
            nc.sync.dma_start(out=bk_s, in_=bk_row)

        for _rep in range(reps):
            s_all = work.tile([128, JTS + IT], F32, name="s_all", tag="s_all")

            def project_group(label, g, src, scol0, w, bsrc, col0):
                # borrow ps_main slots (idle until the main loop) so the
                # scale/transpose pipeline keeps ps_small to itself
                ps = ps_main.tile([128, 512], F32, name=f"psp_{label}{g}",
                                  tag="ps_main")
                for u in range(4):
                    nc.tensor.matmul(
                        ps[:, u * 128:(u + 1) * 128],
                        lhsT=src[:, scol0 + u * 128:scol0 + (u + 1) * 128],
                        rhs=w, start=True, stop=not with_bias)
                    if with_bias:
                        nc.tensor.matmul(ps[:, u * 128:(u + 1) * 128],
                                         lhsT=ones1, rhs=bsrc,
                                         start=False, stop=True)
                rows = work.tile([128, 512], F32, name=f"rows_{label}{g}",
                                 tag=f"rows_{label}{g}")
                sq = scratch.tile([128, 512], F32, name=f"sq_{label}{g}",
                                  tag="sq_scr")
                # all psum->rows copies on ACT (Copy is table-set-free and
                # ACT has prefix slack; DVE is the prefix-critical engine);
                # squares on GpSimd, off ACT to avoid Square<->Sqrt set
                # thrash with the q-side norm chain running concurrently
                nc.scalar.activation(rows, ps, COPYF)
                nc.gpsimd.tensor_mul(sq, rows, rows)
                nc.vector.reduce_sum(
                    s_all[:, col0:col0 + 4],
                    sq.rearrange("p (a b) -> p a b", b=128),
                    axis=AX_X, op=ADD)
                return rows

            def scale_transpose(label, g, rows, mult_src, col0, dstTh,
                                dcol0):
                sc = scratch.tile([128, 512], F32, name=f"sc_{label}{g}",
                                  tag="kn_sc")
                nc.vector.tensor_tensor(
                    sc.rearrange("p (a b) -> p a b", b=128),
                    rows.rearrange("p (a b) -> p a b", b=128),
                    _bcast4(mult_src, col0), MUL)
                ps = ps_small.tile([128, 512], F32, name=f"pst_{label}{g}",
                                   tag="ps_small")
                for u in range(4):
                    nc.tensor.transpose(ps[:, u * 128:(u + 1) * 128],
                                        sc[:, u * 128:(u + 1) * 128], ident)
                nc.scalar.activation(dstTh[:, dcol0:dcol0 + 512], ps, COPYF)

            def bhl_half(h, b_h):
                """b (cols 16h..16h+15) -> bf16 hi/lo rows of bhl[:, 2048h:]."""
                bhi16 = work.tile([128, 16], BF16, name=f"bhi16_{h}")
                nc.vector.tensor_copy(bhi16, b_h)
                bhi32 = work.tile([128, 16], F32, name=f"bhi32_{h}")
                nc.vector.tensor_copy(bhi32, bhi16)
                blo32 = work.tile([128, 16], F32, name=f"blo32_{h}")
                nc.vector.tensor_sub(blo32, b_h, bhi32)
                for src, row, nm in ((bhi32, 0, "hi"), (blo32, 1, "lo")):
                    pst = ps_small.tile([16, 128], F32, name=f"psb_{nm}{h}",
                                        tag="ps_small")
                    nc.tensor.transpose(pst, src, ident)
                    sb16 = work.tile([16, 128], BF16, name=f"sb16_{nm}{h}")
                    nc.vector.tensor_copy(sb16, pst)
                    nc.sync.dma_start(out=bhl[row:row + 1,
                                              2048 * h:2048 * (h + 1)], in_=sb16)

            # ---- q side (unblocks qsT for the main loop) --------------------
            qsT = work.tile([D, R], F32, name="qsT", tag="qsT")
            bhl = work.tile([2, B], BF16, name="bhl", tag="bhl")
            ksT2 = work.tile([D, B], F32, name="ksT2", tag="ksT2")

            r_tiles = []
            for it in range(IT):
                r_tiles.append(work.tile([128, B], F32, name=f"r{it}",
                                         tag=f"r{it}"))
            rowtot = work.tile([128, IT], F32, name="rowtot", tag="rowtot")

            def main_chunk(it, ci):
                col0, width = CHUNKS[ci]
                ps = ps_main.tile([128, 1536], F32, name=f"pm{it}_{ci}",
                                  tag="ps_main")
                isl = slice(it * 128, (it + 1) * 128)
                for u in range(width // 512):
                    lo = col0 + u * 512
                    pslice = ps[:, u * 512:(u + 1) * 512]
                    nc.tensor.matmul(pslice, lhsT=qsT[:, isl],
                                     rhs=ksT2[:, lo:lo + 512],
                                     start=True, stop=False)
                    nc.tensor.matmul(pslice, lhsT=ones2,
                                     rhs=bhl[:, lo:lo + 512],
                                     start=False, stop=True)
                rt = r_tiles[it]
                nc.scalar.activation(rt[:, col0:col0 + width], ps[:, 0:width],
                                     SQRT, bias=a_q[:, it:it + 1])
                nc.gpsimd.tensor_scalar_add(rt[:, col0:col0 + width],
                                            rt[:, col0:col0 + width], 1.0)
                nc.vector.reciprocal(rt[:, col0:col0 + width],
                                     rt[:, col0:col0 + width])

            # all projections up front: PE stream has no stalls, trios trail on
            # DVE/ACT/Pool
            q_rows = project_group("q", 0, qT_s, 0, wq_s,
                                   bq_s if with_bias else None, JTS)
            k_rows = []
            for g in range(KG):
                k_rows.append(project_group(
                    "k", g, kT_s, g * 512, wk_s,
                    bk_s if with_bias else None, 4 * g))

            # q chain early (overlaps k projections), then one combined k chain
            u_q, a_q = _norm_chain(nc, work, s_all[:, JTS:JTS + IT], IT, cD1, "q")
            scale_transpose("q", 0, q_rows, u_q, 0, qsT, 0)

            u_k, b_k = _norm_chain(nc, work, s_all[:, 0:JTS], JTS, cD1, "k")
            vm2 = work.tile([128, JTS], F32, name="vm2", tag="vm2")
            nc.vector.tensor_scalar_mul(vm2, u_k, -2.0)

            # b hi/lo split + transpose into the [2,B] ext-row tile
            bhi16 = work.tile([128, JTS], BF16, name="bhi16", tag="bhi16")
            nc.vector.tensor_copy(bhi16, b_k)
            bhi32 = work.tile([128, JTS], F32, name="bhi32", tag="bhi32")
            nc.vector.tensor_copy(bhi32, bhi16)
            blo32 = work.tile([128, JTS], F32, name="blo32", tag="blo32")
            nc.vector.tensor_sub(blo32, b_k, bhi32)
            for src_, row, nm in ((bhi32, 0, "hi"), (blo32, 1, "lo")):
                pst = ps_small.tile([JTS, 128], F32, name=f"psb_{nm}",
                                    tag="ps_small")
                nc.tensor.transpose(pst, src_, ident)
                sb16 = work.tile([JTS, 128], BF16, name=f"sb16_{nm}", tag=f"sb16_{nm}")
                nc.vector.tensor_copy(sb16, pst)
                nc.sync.dma_start(out=bhl[row:row + 1, :], in_=sb16)

            for g in range(3):
                scale_transpose("k", g, k_rows[g], vm2, 4 * g, ksT2, 512 * g)
            for it in range(IT):
                main_chunk(it, 0)                  # cols 0-1535: groups 0-2
            for g in range(3, KG):
                scale_transpose("k", g, k_rows[g], vm2, 4 * g, ksT2, 512 * g)

            # ---- per-i-tile: remaining chunks, exp, row-normalize, store ----
            # Folding exp into the main loop costs two ACT table reloads per
            # i-tile (sqrt<->exp), but ACT has slack and the output DMA stream
            # (the 23us bandwidth floor) starts ~30us earlier.
            for pair in ((0, 1), (2, 3)):
                for it in pair:
                    main_chunk(it, 1)
                    main_chunk(it, 2)
                for it in pair:
                    rt = r_tiles[it]
                    nc.scalar.activation(rt, rt, EXPF, scale=TEMP,
                                         accum_out=rowtot[:, it:it + 1])
                    inv = work.tile([128, 1], F32, name=f"inv{it}",
                                    tag=f"inv{it}")
                    nc.vector.reciprocal(inv, rowtot[:, it:it + 1])
                    for mh in range(2):
                        nc.vector.tensor_scalar_mul(
                            rt[:, mh * 2048:(mh + 1) * 2048],
                            rt[:, mh * 2048:(mh + 1) * 2048], inv[:, 0:1])
                    # split across HWDGE queues: one dma_start uses a single
                    # ~31GB/s queue; concurrent ones stripe the write
                    nsp = OUT_DMA_SPLIT
                    w = B // nsp
                    for dq in range(nsp):
                        nc.sync.dma_start(
                            out=out[it * 128:(it + 1) * 128,
                                    dq * w:(dq + 1) * w],
                            in_=rt[:, dq * w:(dq + 1) * w])
    return nc


_NC_CACHE = {}


def _get_nc(with_bias, reps=1):
    key = (with_bias, reps, OUT_DMA_SPLIT)
    if key not in _NC_CACHE:
        from concourse import bacc
        nc = bacc.Bacc("TRN2", target_bir_lowering=False, debug=False)
        _trace(nc, with_bias, reps=reps)
        nc.compile()
        _NC_CACHE[key] = nc
    return _NC_CACHE[key]


def _in_maps(query_points, key_points, Wq, bq, Wk, bk, with_bias):
    qT = np.ascontiguousarray(query_points.T.astype(np.float32, copy=False))
    kT = np.ascontiguousarray(key_points.T.astype(np.float32, copy=False))
    wqT = np.ascontiguousarray(Wq.T.astype(np.float32, copy=False))
    wkT = np.ascontiguousarray(Wk.T.astype(np.float32, copy=False))
    maps = []
    for c in range(NCORES):
        m = {
            "qT": np.ascontiguousarray(qT[:, c * R:(c + 1) * R]),
            "kT": kT,
            "wqT": wqT,
            "wkT": wkT,
        }
        if with_bias:
            m["bq_row"] = np.ascontiguousarray(
                bq.astype(np.float32, copy=False).reshape(1, D))
            m["bk_row"] = np.ascontiguousarray(
                bk.astype(np.float32, copy=False).reshape(1, D))
        maps.append(m)
    return maps


LAST_EXEC_NS = None


def run(query_points, key_points, Wq, bq, Wk, bk, trace=False):
    global LAST_EXEC_NS
    query_points = np.asarray(query_points, dtype=np.float32)
    key_points = np.asarray(key_points, dtype=np.float32)
    Wq = np.asarray(Wq, dtype=np.float32)
    bq = np.asarray(bq, dtype=np.float32)
    Wk = np.asarray(Wk, dtype=np.float32)
    bk = np.asarray(bk, dtype=np.float32)
    with_bias = bool(np.any(bq) or np.any(bk))
    nc = _get_nc(with_bias)
    maps = _in_maps(query_points, key_points, Wq, bq, Wk, bk, with_bias)
    from concourse import bass_utils
    res = bass_utils.run_bass_kernel_spmd(
        nc, maps, core_ids=list(range(NCORES)), trace=trace)
    LAST_EXEC_NS = res.exec_time_ns
    out = np.concatenate([res.results[c]["out"] for c in range(NCORES)], axis=0)
    return out


def kernel(query_points, key_points, Wq, bq, Wk, bk):
    return run(query_points, key_points, Wq, bq, Wk, bk, trace=False)



# revision 2
# speedup vs baseline: 5.4043x; 5.4043x over previous
"""Trainium2 Bass kernel for nn_MockAttentionHead (v2).

Math (validated in numcheck.py, absmax-rel 1.8e-3 vs 2e-2 gate):
  out = exp(temp/(1+sqrt(d2))) row-normalized,
  d2 = a_i + b_j - 2 qn_i.kn_j  with the metric norms reduced analytically:
  s=|q|^2, t=s/D+1, fro=sqrt(t^2+D-1), norm=sqrt(s*t/fro), a=fro/t.

v2 design vs v1:
- all matmuls bf16 (1 PE pass vs 4 for fp32)
- projections computed TWICE: row-layout (for norms) and transposed
  (Wq @ xT directly) -- kills all 36 PE transposes and the ACT copies
- a_i and b_j folded into PSUM via a K=4 bf16 matmul (hi/lo bf16 pairs)
- elementwise chain: ACT sqrt (psum->sbuf f32), +1 on Pool/DVE,
  reciprocal_approx_fast on DVE (fp32, ~51 ULP), ACT exp->f16 + accum,
  f16 row-scale, f16 output DMA (host upcasts)
- ACT table sets: all Sqrt ops batched, then all Exp ops: 2 loads total
- engine balance: squares on ACT (Square is in every table set),
  grouped norm-reduces on Pool, +1 on Pool (chunks 0,1) / DVE (chunk 2)

Sharding: data-parallel over query rows; 512 rows/core, key side
replicated. No collectives.
"""

import sys
import numpy as np

sys.path.insert(0, "/opt/trn_rl_repo")

import concourse.bass as bass
import concourse.mybir as mybir
import concourse.tile as tile
from concourse.masks import make_identity

B = 4096
D = 128
NCORES = 8
R = B // NCORES          # 512 query rows per core
IT = R // 128            # 4 i-tiles per core
JTS = B // 128           # 32 j-tiles
KG = 8                   # k groups of 512 points
NG = KG + 1              # + 1 q group
CHUNKS = [(0, 1536), (1536, 1536), (3072, 1024)]
TEMP = float(np.sqrt(float(D)))
OUT_DMA_SPLIT = 2

F32 = mybir.dt.float32
F16 = mybir.dt.float16
BF16 = mybir.dt.bfloat16
MUL = mybir.AluOpType.mult
ADD = mybir.AluOpType.add
SUB = mybir.AluOpType.subtract
AX_X = mybir.AxisListType.X
SQRT = mybir.ActivationFunctionType.Sqrt
EXPF = mybir.ActivationFunctionType.Exp
SQUARE = mybir.ActivationFunctionType.Square


def _brow(src, col0, n, parts=128):
    """AP over src[0, col0:col0+n] broadcast across `parts` partitions."""
    return bass.AP(tensor=src.tensor, offset=src.offset + col0,
                   ap=[[0, parts], [1, n]])


def _norm_chain(nc, pool, s, n, cD1, label):
    """u = 1/metric-norm and a = ||xn||^2 from packed row-norm tile s."""
    t = pool.tile([128, n], F32, name=f"t_{label}", tag=f"t_{label}")
    nc.vector.tensor_scalar(t, s, 1.0 / D, 1.0, MUL, ADD)
    t2 = pool.tile([128, n], F32, name=f"t2_{label}", tag=f"t2_{label}")
    nc.vector.tensor_mul(t2, t, t)
    fro = pool.tile([128, n], F32, name=f"fro_{label}", tag=f"fro_{label}")
    nc.scalar.activation(fro, t2, SQRT, bias=cD1[:, 0:1])
    rec = pool.tile([128, n], F32, name=f"rec_{label}", tag=f"rec_{label}")
    nc.vector.reciprocal(rec, fro)
    rt_ = pool.tile([128, n], F32, name=f"rt_{label}", tag=f"rt_{label}")
    nc.vector.reciprocal(rt_, t)
    a = pool.tile([128, n], F32, name=f"a_{label}", tag=f"a_{label}")
    nc.vector.tensor_mul(a, fro, rt_)
    num = pool.tile([128, n], F32, name=f"num_{label}", tag=f"num_{label}")
    nc.vector.tensor_mul(num, s, t)
    nc.vector.tensor_mul(num, num, rec)
    qn = pool.tile([128, n], F32, name=f"qn_{label}", tag=f"qn_{label}")
    nc.scalar.activation(qn, num, SQRT)
    u = pool.tile([128, n], F32, name=f"u_{label}", tag=f"u_{label}")
    nc.vector.reciprocal(u, qn)
    return u, a


def _trace(nc, with_bias, reps=1):
    from contextlib import ExitStack

    qT = nc.dram_tensor("qT", [D, R], F32, kind="ExternalInput").ap()
    kT = nc.dram_tensor("kT", [D, B], F32, kind="ExternalInput").ap()
    wqT = nc.dram_tensor("wqT", [D, D], F32, kind="ExternalInput").ap()
    wkT = nc.dram_tensor("wkT", [D, D], F32, kind="ExternalInput").ap()
    if with_bias:
        bq_row = nc.dram_tensor("bq_row", [1, D], F32, kind="ExternalInput").ap()
        bk_row = nc.dram_tensor("bk_row", [1, D], F32, kind="ExternalInput").ap()
    out = nc.dram_tensor("out", [R, B], F16, kind="ExternalOutput").ap()

    with tile.TileContext(nc) as tc, ExitStack() as ctx:
        ctx.enter_context(nc.allow_low_precision(
            reason="bf16 matmuls / f16 scores validated vs fp32 ref at 1.8e-3"
        ))
        consts = ctx.enter_context(tc.tile_pool(name="consts", bufs=1))
        work = ctx.enter_context(tc.tile_pool(name="work", bufs=1))
        scratch = ctx.enter_context(tc.tile_pool(name="scratch", bufs=3))
        ps_small = ctx.enter_context(
            tc.tile_pool(name="ps_small", bufs=2, space="PSUM"))
        ps_main = ctx.enter_context(
            tc.tile_pool(name="ps_main", bufs=2, space="PSUM"))

        ident = consts.tile([128, 128], F32, name="ident")
        make_identity(nc, ident)
        cD1 = consts.tile([128, 1], F32, name="cD1")
        nc.vector.memset(cD1, float(D - 1))
        # dummy sqrt: pulls the Sqrt table set in at t~0 (it also contains
        # Square, so squares/chain/main sqrts all run with zero reloads)
        warm = consts.tile([128, 1], F32, name="warm")
        nc.scalar.activation(warm, cD1, SQRT)

        # ---- input DMAs: weights first (tiny, unblock projections) ----
        wk_s = consts.tile([D, D], F32, name="wk_s")
        nc.sync.dma_start(out=wk_s, in_=wkT)
        wq_s = consts.tile([D, D], F32, name="wq_s")
        nc.sync.dma_start(out=wq_s, in_=wqT)
        kT_s = consts.tile([D, B], F32, name="kT_s")
        nc.sync.dma_start(out=kT_s[:, 0:1024], in_=kT[:, 0:1024])
        qT_s = consts.tile([D, R], F32, name="qT_s")
        nc.sync.dma_start(out=qT_s, in_=qT)
        for h in range(1, 4):
            nc.sync.dma_start(out=kT_s[:, h * 1024:(h + 1) * 1024],
                              in_=kT[:, h * 1024:(h + 1) * 1024])
        if with_bias:
            bq_s = consts.tile([1, D], F32, name="bq_s")
            nc.sync.dma_start(out=bq_s, in_=bq_row)
            bk_s = consts.tile([1, D], F32, name="bk_s")
            nc.sync.dma_start(out=bk_s, in_=bk_row)

        for _rep in range(reps):
            foldR = work.tile([4, B], BF16, name="foldR", tag="foldR")
            nc.vector.memset(foldR, 1.0)
            foldL_all = work.tile([4, R], BF16, name="foldL_all",
                                  tag="foldL_all")
            nc.vector.memset(foldL_all, 1.0)
            # ---- bf16 copies (kT per chunk so rows-proj starts early) ----
            wk_bf = work.tile([D, D], BF16, name="wk_bf", tag="wk_bf")
            nc.vector.tensor_copy(wk_bf, wk_s)
            wq_bf = work.tile([D, D], BF16, name="wq_bf", tag="wq_bf")
            nc.vector.tensor_copy(wq_bf, wq_s)
            kT_bf = work.tile([D, B], BF16, name="kT_bf", tag="kT_bf")
            nc.vector.tensor_copy(kT_bf[:, 0:1024], kT_s[:, 0:1024])
            qT_bf = work.tile([D, R], BF16, name="qT_bf", tag="qT_bf")
            nc.vector.tensor_copy(qT_bf, qT_s)
            for h in range(1, 4):
                nc.vector.tensor_copy(kT_bf[:, h * 1024:(h + 1) * 1024],
                                      kT_s[:, h * 1024:(h + 1) * 1024])
            if with_bias:
                bq_bf = work.tile([1, D], BF16, name="bq_bf", tag="bq_bf")
                nc.vector.tensor_copy(bq_bf, bq_s)
                bk_bf = work.tile([1, D], BF16, name="bk_bf", tag="bk_bf")
                nc.vector.tensor_copy(bk_bf, bk_s)
                ones1_bf = work.tile([1, B], BF16, name="ones1_bf",
                                     tag="ones1_bf")
                nc.vector.memset(ones1_bf, 1.0)

            # s_all cols: 0..31 k-groups, 32..35 q i-tiles
            s_all = work.tile([128, JTS + IT], F32, name="s_all", tag="s_all")

            knT_bf = work.tile([D, B], BF16, name="knT_bf", tag="knT_bf")
            qnT_bf = work.tile([D, R], BF16, name="qnT_bf", tag="qnT_bf")

            def prep_norms(label, g, xbf, wbf, bbf, col0):
                """rows-proj -> squares(ACT) -> grouped reduce(DVE)."""
                psr = ps_small.tile([128, 512], F32, name=f"psr_{label}{g}",
                                    tag="ps_small")
                for u in range(4):
                    nc.tensor.matmul(
                        psr[:, u * 128:(u + 1) * 128],
                        lhsT=xbf[:, g * 512 + u * 128:g * 512 + (u + 1) * 128],
                        rhs=wbf, start=True, stop=not with_bias)
                    if with_bias:
                        nc.tensor.matmul(psr[:, u * 128:(u + 1) * 128],
                                         lhsT=ones1_bf[:, 0:128], rhs=bbf,
                                         start=False, stop=True)
                sq = scratch.tile([128, 512], F32, name=f"sq_{label}{g}",
                                  tag="sq_scr")
                nc.scalar.activation(sq, psr, SQUARE)
                nc.vector.reduce_sum(
                    s_all[:, col0:col0 + 4],
                    sq.rearrange("p (a b) -> p a b", b=128),
                    axis=AX_X, op=ADD)

            # prep order: k0,k1,q,k2,k3 feed chain A; k4..k7 feed chain B.
            # s_all layout: cols 0:4 q, col 4+4g for k group g.
            prep_norms("k", 0, kT_bf, wk_bf,
                       bk_bf if with_bias else None, 4)
            prep_norms("k", 1, kT_bf, wk_bf,
                       bk_bf if with_bias else None, 8)
            prep_norms("q", 0, qT_bf, wq_bf,
                       bq_bf if with_bias else None, 0)
            for g in range(2, 4):
                prep_norms("k", g, kT_bf, wk_bf,
                           bk_bf if with_bias else None, 4 + 4 * g)

            vk_row = work.tile([1, B], BF16, name="vk_row", tag="vk_row")
            vk_full = work.tile([128, B], BF16, name="vk_full", tag="vk_full")

            def scale_k_group(g):
                ps_k = ps_small.tile([128, 512], F32, name=f"ps_kT{g}",
                                     tag="ps_small")
                nc.tensor.matmul(ps_k, lhsT=wk_bf,
                                 rhs=kT_bf[:, g * 512:(g + 1) * 512],
                                 start=True, stop=not with_bias)
                if with_bias:
                    nc.tensor.matmul(ps_k, lhsT=bk_bf,
                                     rhs=ones1_bf[:, 0:512],
                                     start=False, stop=True)
                nc.vector.tensor_tensor(
                    knT_bf[:, g * 512:(g + 1) * 512], ps_k,
                    vk_full[:, g * 512:(g + 1) * 512], MUL)

            # ---- chain A: q + k0..k3 (cols 0:20) ----
            uA, aA = _norm_chain(nc, work, s_all[:, 0:20], 20, cD1, "A")
            # B-side rows/squares/reduces early so its chain isn't gated on
            # the A dance
            for g in range(4, KG):
                prep_norms("k", g, kT_bf, wk_bf,
                           bk_bf if with_bias else None, 4 + 4 * g)
            NKA = 16  # k cols in A
            # combA: 0:4 a_hi, 4:8 a_lo, 8:12 u_q, 12:28 bhiA, 28:44 bloA,
            # 44:60 vmA
            combA = work.tile([128, 60], F32, name="combA", tag="combA")
            a_q = aA[:, 0:IT]
            ahi_bf = work.tile([128, IT], BF16, name="ahi_bf", tag="ahi_bf")
            nc.vector.tensor_copy(ahi_bf, a_q)
            nc.vector.tensor_copy(combA[:, 0:4], ahi_bf)
            nc.vector.tensor_tensor(combA[:, 4:8], a_q, combA[:, 0:4], SUB)
            nc.vector.tensor_copy(combA[:, 8:12], uA[:, 0:IT])
            bhiA_bf = work.tile([128, NKA], BF16, name="bhiA_bf",
                                tag="bhiA_bf")
            nc.vector.tensor_copy(bhiA_bf, aA[:, 4:20])
            nc.vector.tensor_copy(combA[:, 12:28], bhiA_bf)
            nc.vector.tensor_tensor(combA[:, 28:44], aA[:, 4:20],
                                    combA[:, 12:28], SUB)
            nc.vector.tensor_scalar_mul(combA[:, 44:60], uA[:, 4:20], -2.0)

            psA = ps_small.tile([60, 128], F32, name="psA", tag="ps_small")
            nc.tensor.transpose(psA, combA, ident)
            # one bulk bf16 copy (partition 0) frees the psum slot; all row
            # extraction below is DMA (engines can't address partition
            # offsets, DMA can)
            sbA = work.tile([60, 128], BF16, name="sbA", tag="sbA")
            nc.vector.tensor_copy(sbA, psA)

            # q side first: uq broadcast is tiny and unblocks qnT (the lhsT
            # of every main matmul)
            uq_row = work.tile([1, R], BF16, name="uq_row", tag="uq_row")
            nc.sync.dma_start(out=uq_row, in_=sbA[8:12, :])
            uq_full = work.tile([128, R], BF16, name="uq_full", tag="uq_full")
            nc.gpsimd.partition_broadcast(uq_full, uq_row)

            nc.sync.dma_start(out=vk_row[:, 0:2048], in_=sbA[44:60, :])
            for h in range(2):
                nc.gpsimd.partition_broadcast(
                    vk_full[:, h * 1024:(h + 1) * 1024],
                    vk_row[:, h * 1024:(h + 1) * 1024])

            nc.scalar.dma_start(out=foldL_all[2:3, :], in_=sbA[0:4, :])
            nc.scalar.dma_start(out=foldL_all[3:4, :], in_=sbA[4:8, :])
            nc.scalar.dma_start(out=foldR[0:1, 0:2048], in_=sbA[12:28, :])
            nc.scalar.dma_start(out=foldR[1:2, 0:2048], in_=sbA[28:44, :])

            ps_q = ps_small.tile([128, 512], F32, name="ps_qT", tag="ps_small")
            nc.tensor.matmul(ps_q, lhsT=wq_bf, rhs=qT_bf,
                             start=True, stop=not with_bias)
            if with_bias:
                nc.tensor.matmul(ps_q, lhsT=bq_bf, rhs=ones1_bf[:, 0:512],
                                 start=False, stop=True)
            nc.vector.tensor_tensor(qnT_bf, ps_q, uq_full, MUL)

            for g in range(4):
                scale_k_group(g)

            # ---- chain B: k4..k7 (cols 20:36) ----
            uB, aB = _norm_chain(nc, work, s_all[:, 20:36], 16, cD1, "B")
            combB = work.tile([128, 48], F32, name="combB", tag="combB")
            bhiB_bf = work.tile([128, 16], BF16, name="bhiB_bf",
                                tag="bhiB_bf")
            nc.vector.tensor_copy(bhiB_bf, aB)
            nc.vector.tensor_copy(combB[:, 0:16], bhiB_bf)
            nc.vector.tensor_tensor(combB[:, 16:32], aB, combB[:, 0:16], SUB)
            nc.vector.tensor_scalar_mul(combB[:, 32:48], uB, -2.0)

            psB = ps_small.tile([48, 128], F32, name="psB", tag="ps_small")
            nc.tensor.transpose(psB, combB, ident)
            sbB = work.tile([48, 128], BF16, name="sbB", tag="sbB")
            nc.vector.tensor_copy(sbB, psB)

            nc.sync.dma_start(out=vk_row[:, 2048:4096], in_=sbB[32:48, :])
            for h in range(2, 4):
                nc.gpsimd.partition_broadcast(
                    vk_full[:, h * 1024:(h + 1) * 1024],
                    vk_row[:, h * 1024:(h + 1) * 1024])
            nc.scalar.dma_start(out=foldR[0:1, 2048:4096], in_=sbB[0:16, :])
            nc.scalar.dma_start(out=foldR[1:2, 2048:4096], in_=sbB[16:32, :])

            # ---- main loop ----
            s_tiles = []
            for it in range(IT):
                s_tiles.append(work.tile([128, B], F32, name=f"s{it}",
                                         tag=f"s{it}"))
            e_tiles = []
            for it in range(IT):
                e_tiles.append(work.tile([128, B], F16, name=f"e{it}",
                                         tag=f"e{it}"))
            rowtot = work.tile([128, IT], F32, name="rowtot", tag="rowtot")

            # exp's scale comes from temp_col, which is written only after the
            # last sqrt -- forces every Exp behind every Sqrt in the ACT
            # queue, so the table set loads exactly twice.
            tempc = work.tile([128, 1], F32, name="tempc", tag="tempc")
            nc.vector.memset(tempc, TEMP)
            temp_col = work.tile([128, 1], F32, name="temp_col",
                                 tag="temp_col")

            def chunk_work(it, ci, defer=False):
                col0, width = CHUNKS[ci]
                isl = slice(it * 128, (it + 1) * 128)
                st = s_tiles[it]
                ps = ps_main.tile([128, 1536], F32, name=f"pm{it}_{ci}",
                                  tag="ps_main")
                for u in range(width // 512):
                    lo = col0 + u * 512
                    pslice = ps[:, u * 512:(u + 1) * 512]
                    nc.tensor.matmul(pslice, lhsT=qnT_bf[:, isl],
                                     rhs=knT_bf[:, lo:lo + 512],
                                     start=True, stop=False)
                    nc.tensor.matmul(
                        pslice, lhsT=foldL_all[:, it * 128:(it + 1) * 128],
                        rhs=foldR[:, lo:lo + 512],
                        start=False, stop=True)
                nc.scalar.activation(st[:, col0:col0 + width],
                                     ps[:, 0:width], SQRT)
                if defer:
                    return
                # +1: wide chunks on Pool, short c2 on DVE
                if ci < 2:
                    nc.gpsimd.tensor_scalar_add(
                        st[:, col0:col0 + width],
                        st[:, col0:col0 + width], 1.0)
                else:
                    nc.vector.tensor_scalar_add(
                        st[:, col0:col0 + width],
                        st[:, col0:col0 + width], 1.0)
                nc.vector.reciprocal_approx_fast(
                    out=st[:, col0:col0 + width],
                    in_=st[:, col0:col0 + width])

            # phase A: chunk 0 for all i-tiles first (needs only chain-A
            # groups, overlaps the B-side prep), then i-major for c1+c2 so
            # i0 completes early and exp can start.
            for g in range(4, KG):
                scale_k_group(g)
            for it in range(IT):
                chunk_work(it, 0)
            for it in range(IT):
                chunk_work(it, 1)
                chunk_work(it, 2, defer=(it == IT - 1))

            # marker: TEMP value, data-dependent on the last sqrt.
            # On Pool: its queue is free the moment the last sqrt lands,
            # while DVE still has recips queued.
            st3 = s_tiles[IT - 1]
            nc.scalar.activation(temp_col, st3[:, B - 1:B],
                                 mybir.ActivationFunctionType.Identity,
                                 bias=tempc[:, 0:1], scale=0.0)
            col0, width = CHUNKS[-1]
            nc.vector.tensor_scalar_add(
                st3[:, col0:col0 + width], st3[:, col0:col0 + width], 1.0)
            nc.vector.reciprocal_approx_fast(
                out=st3[:, col0:col0 + width],
                in_=st3[:, col0:col0 + width])
            # also deferred: i3's c2 was the last sqrt; nothing else pending

            # phase B: exp (one table load), normalize, store
            for it in range(IT):
                st, et = s_tiles[it], e_tiles[it]
                nc.scalar.activation(et, st, EXPF, scale=temp_col[:, 0:1],
                                     accum_out=rowtot[:, it:it + 1])
                inv = work.tile([128, 1], F32, name=f"inv{it}", tag=f"inv{it}")
                nc.vector.reciprocal(inv, rowtot[:, it:it + 1])
                # scale+store in halves: the first DMA streams while the
                # second half is still scaling
                for dq in range(2):
                    half = slice(dq * 2048, (dq + 1) * 2048)
                    nc.vector.tensor_scalar_mul(et[:, half], et[:, half],
                                                inv[:, 0:1])
                    nc.sync.dma_start(
                        out=out[it * 128:(it + 1) * 128, half],
                        in_=et[:, half])
    return nc


_NC_CACHE = {}


def _get_nc(with_bias, reps=1):
    key = (with_bias, reps)
    if key not in _NC_CACHE:
        from concourse import bacc
        nc = bacc.Bacc("TRN2", target_bir_lowering=False, debug=False)
        _trace(nc, with_bias, reps=reps)
        nc.compile()
        _NC_CACHE[key] = nc
    return _NC_CACHE[key]


def _in_maps(query_points, key_points, Wq, bq, Wk, bk, with_bias):
    qT = np.ascontiguousarray(query_points.T.astype(np.float32, copy=False))
    kT = np.ascontiguousarray(key_points.T.astype(np.float32, copy=False))
    wqT = np.ascontiguousarray(Wq.T.astype(np.float32, copy=False))
    wkT = np.ascontiguousarray(Wk.T.astype(np.float32, copy=False))
    maps = []
    for c in range(NCORES):
        m = {
            "qT": np.ascontiguousarray(qT[:, c * R:(c + 1) * R]),
            "kT": kT,
            "wqT": wqT,
            "wkT": wkT,
        }
        if with_bias:
            m["bq_row"] = np.ascontiguousarray(
                bq.astype(np.float32, copy=False).reshape(1, D))
            m["bk_row"] = np.ascontiguousarray(
                bk.astype(np.float32, copy=False).reshape(1, D))
        maps.append(m)
    return maps


LAST_EXEC_NS = None


def run(query_points, key_points, Wq, bq, Wk, bk, trace=False):
    global LAST_EXEC_NS
    query_points = np.asarray(query_points, dtype=np.float32)
    key_points = np.asarray(key_points, dtype=np.float32)
    Wq = np.asarray(Wq, dtype=np.float32)
    bq = np.asarray(bq, dtype=np.float32)
    Wk = np.asarray(Wk, dtype=np.float32)
    bk = np.asarray(bk, dtype=np.float32)
    with_bias = bool(np.any(bq) or np.any(bk))
    nc = _get_nc(with_bias)
    maps = _in_maps(query_points, key_points, Wq, bq, Wk, bk, with_bias)
    from concourse import bass_utils
    res = bass_utils.run_bass_kernel_spmd(
        nc, maps, core_ids=list(range(NCORES)), trace=trace)
    LAST_EXEC_NS = res.exec_time_ns
    out = np.concatenate([res.results[c]["out"] for c in range(NCORES)],
                         axis=0).astype(np.float32)
    return out


def kernel(query_points, key_points, Wq, bq, Wk, bk):
    return run(query_points, key_points, Wq, bq, Wk, bk, trace=False)
